# revision 14
# baseline (speedup 1.0000x reference)
"""Multi-head causal attention (B=8, S=1024, D=768, H=12) on 8 trn2 NeuronCores.

Strategy: data-parallel over batch (one batch element per core, no collectives).

Per-core dataflow (all matmuls fp32r except A@V in bf16):
  - host passes x^T, so Q^T/K^T come from a transposed projection
    (W stationary, x^T moving) and V from a natural projection
    (x^T stationary, W_v moving) -> no on-device transposes at all.
  - attention computed as S^T[k,q] = K @ Q^T per head, with two heads packed
    into the 128-row PE array via row tiling (dh=64).
  - softmax: exp on ScalarE straight out of PSUM (scale 1/8 folded into W_q
    host-side, no max-subtraction needed for these magnitudes); causal mask
    applied as a bf16 0/1 multiply on the few diagonal-crossing blocks;
    the denominator comes for free as row 64 of the A@V matmul by appending
    a ones column to V; division is folded into the PSUM->SBUF copy of the
    A@V result (reciprocal + DMA partition-broadcast via a DRAM scratch).
  - causal block-skip everywhere (upper-triangular blocks never computed).
"""
import sys

if "/opt/trn_rl_repo" not in sys.path:
    sys.path.insert(0, "/opt/trn_rl_repo")

import numpy as np

B, S, D, H = 8, 1024, 768, 12
DH = 64          # head dim
NC_ = 8          # cores
NT = D // 128    # 6 chunks of 128 along D
ST = S // 128    # 8 tiles of 128 along S
QC = S // 512    # 2 chunks of 512 along queries
VPW = H * (DH + 1)  # 780: v' row width (12 heads x (64 + ones col))

_compiled = None


def _build_masks():
    # mask[i, t, j] = 1 if (128*t + i) <= j else 0  (keep condition), bf16
    import ml_dtypes

    i = np.arange(128)[:, None, None]
    t = np.arange(4)[None, :, None]
    j = np.arange(512)[None, None, :]
    m = ((128 * t + i) <= j).astype(np.float32)
    return m.astype(ml_dtypes.bfloat16)


def _build_nc():
    import concourse.bass as bass
    import concourse.mybir as mybir
    import concourse.tile as tile
    from concourse import bacc

    F32 = mybir.dt.float32
    F32R = mybir.dt.float32r
    BF16 = mybir.dt.bfloat16
    AF = mybir.ActivationFunctionType
    MULT = mybir.AluOpType.mult

    nc = bacc.Bacc("TRN2", target_bir_lowering=False, debug=False)

    xT_d = nc.dram_tensor("xT", [D, S], F32, kind="ExternalInput")
    wq_d = nc.dram_tensor("wq", [D, D], F32, kind="ExternalInput")
    wk_d = nc.dram_tensor("wk", [D, D], F32, kind="ExternalInput")
    wv_d = nc.dram_tensor("wv", [D, D], F32, kind="ExternalInput")
    wp_d = nc.dram_tensor("wp", [D, D], F32, kind="ExternalInput")
    mask_d = nc.dram_tensor("masks", [128, 4, 512], BF16, kind="ExternalInput")
    y_d = nc.dram_tensor("y", [S, D], F32, kind="ExternalOutput")
    recip_d = nc.dram_tensor("recip_scratch", [H, QC, 512], F32)

    with tile.TileContext(nc) as tc:
        with (
            tc.tile_pool(name="static", bufs=1) as static,
            tc.tile_pool(name="w", bufs=12) as wpool,
            tc.tile_pool(name="pt", bufs=13) as ptpool,
            tc.tile_pool(name="small", bufs=2) as small,
            tc.tile_pool(name="rbp", bufs=2) as rbp,
            tc.tile_pool(name="y", bufs=2) as ypool,
            tc.tile_pool(name="psb", bufs=2, space="PSUM") as psb,
            tc.tile_pool(name="psproj", bufs=2, space="PSUM") as psproj,
            tc.tile_pool(name="pso", bufs=2, space="PSUM") as pso,
        ):
            # ---- persistent SBUF ----
            xT = static.tile([128, NT, S], F32R)
            qT = static.tile([128, NT, S], F32R)
            kT = static.tile([128, NT, S], F32R)
            vp = static.tile([128, ST, VPW], BF16)
            outT = static.tile([128, NT, S], F32R)
            msk = static.tile([128, 4, 512], BF16)

            for dc in range(NT):
                for h2 in range(2):
                    nc.sync.dma_start(
                        xT[:, dc, 512 * h2:512 * (h2 + 1)],
                        xT_d[128 * dc:128 * (dc + 1), 512 * h2:512 * (h2 + 1)].bitcast(F32R))
            nc.sync.dma_start(msk[:], mask_d[:])
            nc.vector.memset(vp[:], 1.0)

            # ---- stage C (v' = x @ W_v + ones cols) as work units ----
            wv_t = []
            for dc in range(NT):
                w = wpool.tile([128, D], F32R, tag="w")
                nc.sync.dma_start(w[:], wv_d[128 * dc:128 * (dc + 1), :].bitcast(F32R))
                wv_t.append(w)

            def emit_c_unit(st, half):
                # half 0: heads 0..7 (cols 0:512); half 1: heads 8..11 (512:768)
                lo, hi = (0, 512) if half == 0 else (512, 768)
                ps = psproj.tile([128, 512], F32, tag="proj")
                w_ = hi - lo
                for dc in range(NT):
                    nc.tensor.matmul(
                        ps[:, 0:w_], xT[:, dc, 128 * st:128 * (st + 1)],
                        wv_t[dc][:, lo:hi], start=(dc == 0), stop=(dc == NT - 1))
                dst = vp[:, st, :].rearrange("p (h e) -> p h e", e=DH + 1)
                h0 = 8 * half
                h1 = 12 if half else 8
                nc.vector.tensor_copy(
                    out=dst[:, h0:h1, 0:DH],
                    in_=ps[:, 0:w_].rearrange("p (h d) -> p h d", d=DH))

            # ---- interleaved: per head-pair hp: project qT/kT chunk, attention ----
            def emit_proj_unit(w_tiles, nt, dst, sc):
                # dst[:, nt, 512sc:+512] = (W[:, 128nt:+128]).T @ xT[:, :, 512sc:+512]
                ps = psproj.tile([128, 512], F32, tag="proj")
                for dc in range(NT):
                    nc.tensor.matmul(
                        ps[:],
                        w_tiles[dc][:, 128 * nt:128 * (nt + 1)],
                        xT[:, dc, 512 * sc:512 * (sc + 1)],
                        start=(dc == 0), stop=(dc == NT - 1))
                nc.vector.tensor_copy(out=dst[:, nt, 512 * sc:512 * (sc + 1)], in_=ps[:])

            # projection work units, interleaved into the ACT-bound attention
            # stream so the PE fills exp-wait gaps with projection matmuls
            proj_units = []

            def pop_unit():
                if proj_units:
                    hp_u, fn = proj_units.pop(0)
                    fn()

            def drain_units(hp_limit):
                while proj_units and proj_units[0][0] <= hp_limit:
                    hp_u, fn = proj_units.pop(0)
                    fn()

            wq_t, wk_t = [], []
            for dc in range(NT):
                w = wpool.tile([128, D], F32R, tag="w")
                nc.sync.dma_start(w[:], wq_d[128 * dc:128 * (dc + 1), :].bitcast(F32R))
                wq_t.append(w)
            for dc in range(NT):
                w = wpool.tile([128, D], F32R, tag="w")
                nc.sync.dma_start(w[:], wk_d[128 * dc:128 * (dc + 1), :].bitcast(F32R))
                wk_t.append(w)
            wp_t = []
            for dc in range(NT):
                w = wpool.tile([128, D], F32R, tag="w")
                nc.sync.dma_start(w[:], wp_d[128 * dc:128 * (dc + 1), :].bitcast(F32R))
                wp_t.append(w)

            def emit_e_unit(hp, st):
                # y[st] += outT[:, hp] chunk  @ wp rows of this head-pair
                ps = psproj.tile([128, 512], F32, tag="proj")
                nc.tensor.matmul(ps[:], outT[:, hp, 128 * st:128 * (st + 1)],
                                 wp_t[hp][:, 0:512], start=True, stop=True)
                ps2 = psproj.tile([128, 512], F32, tag="proj")
                nc.tensor.matmul(ps2[:, 0:256], outT[:, hp, 128 * st:128 * (st + 1)],
                                 wp_t[hp][:, 512:768], start=True, stop=True)
                y_sb = ypool.tile([128, D], F32, tag="y")
                nc.vector.tensor_copy(out=y_sb[:, 0:512], in_=ps[:])
                nc.vector.tensor_copy(out=y_sb[:, 512:768], in_=ps2[:, 0:256])
                op = mybir.AluOpType.bypass if hp == 0 else mybir.AluOpType.add
                nc.gpsimd.dma_start(
                    y_d[128 * st:128 * (st + 1), :], y_sb[:], accum_op=op)

            # unit queue: (order_tag, fn); popped in order as PE filler
            for st in range(ST):
                for half in range(2):
                    proj_units.append((0 if st < 4 else 0.5,
                                       (lambda st=st, half=half: emit_c_unit(st, half))))
            for hp in range(NT):
                for sc in range(2):
                    proj_units.append((hp, (lambda nt=hp, sc=sc: emit_proj_unit(wq_t, nt, qT, sc))))
                    proj_units.append((hp, (lambda nt=hp, sc=sc: emit_proj_unit(wk_t, nt, kT, sc))))

            # ---- D: attention over (hp, qc) blocks, software-pipelined ----
            # Block i emits its S^T tile groups (feeding ACT's exp stream),
            # then flushes block i-1's deferred A@V slices + normalization.
            # This keeps ACT continuously fed and the PE dense (projection
            # units weave in as filler).  S^T matmuls interleave the two
            # head-halves (A0 B0 A1 B1) so disjoint row-groups overlap.
            blocks = [(hp, qc) for hp in range(NT) for qc in range(QC)]
            pending = []  # deferred OUT closures of the previous block

            def flush_pending():
                while pending:
                    pending.pop(0)()

            for bi, (hp, qc) in enumerate(blocks):
                if qc == 0:
                    drain_units(hp)  # qT/kT for this hp must be emitted
                K = 4 * (qc + 1)
                prev = pending
                pending = []
                pts_all = []
                for kp in range(K // 2):
                    psA = psb.tile([128, 1024], F32, tag="big")
                    psB = psb.tile([128, 1024], F32, tag="big")
                    for j in range(2):
                        kc = 2 * kp + j
                        for hh, ps in ((0, psA), (1, psB)):
                            rows = slice(64 * hh, 64 * (hh + 1))
                            nc.tensor.matmul(
                                ps[:, 512 * j:512 * (j + 1)],
                                kT[rows, hp, 128 * kc:128 * (kc + 1)],
                                qT[rows, hp, 512 * qc:512 * (qc + 1)],
                                start=True, stop=True,
                                tile_position=(64 * hh, 0))
                    pts_pair = []
                    for hh, ps in ((0, psA), (1, psB)):
                        pt = ptpool.tile([128, 1024], BF16, tag="pt")
                        nc.scalar.activation(pt[:], ps[:], AF.Exp)
                        for j in range(2):
                            t = 2 * kp + j - 4 * qc
                            if 0 <= t <= 3:
                                half = pt[:, 512 * j:512 * (j + 1)]
                                nc.vector.tensor_tensor(half, half, msk[:, t, :], MULT)
                        pts_pair.append(pt)
                    pts_all.append(pts_pair)
                    # flush one slice of the previous block between ST groups
                    if prev:
                        prev.pop(0)()
                    pop_unit()
                    if bi < 2:
                        pop_unit()

                po_blk = {}

                def out_slice(hp=hp, qc=qc, K=K, pts_all=pts_all, po_blk=po_blk, kp=None):
                    if kp == 0:
                        po_a = pso.tile([65, 512], F32, tag="po")
                        po_b = pso.tile([65, 512], F32, tag="po")
                        po_blk[0] = (po_a, po_b)
                    poA, poB = po_blk[0]
                    for hh, po in ((0, poA), (1, poB)):
                        h = 2 * hp + hh
                        for j in range(2):
                            kc = 2 * kp + j
                            nc.tensor.matmul(
                                po[:],
                                vp[:, kc, 65 * h:65 * (h + 1)],
                                pts_all[kp][hh][:, 512 * j:512 * (j + 1)],
                                start=(kc == 0), stop=(kc == K - 1))
                    if kp == K // 2 - 1:
                        for hh, po in ((0, poA), (1, poB)):
                            h = 2 * hp + hh
                            rows = slice(64 * hh, 64 * (hh + 1))
                            den = small.tile([1, 512], F32, tag="den")
                            nc.vector.tensor_copy(out=den[:], in_=po[64:65, :])
                            rc = small.tile([1, 512], F32, tag="rc")
                            nc.vector.reciprocal_approx_fast(out=rc[:], in_=den[:])
                            nc.sync.dma_start(recip_d[h, qc, :], rc[:])
                            rb = rbp.tile([64, 512], F32, tag="rb")
                            sl = recip_d[h, qc, :]
                            bc_ap = bass.AP(tensor=sl.tensor, offset=sl.offset,
                                            ap=[[0, 64]] + list(sl.ap))
                            nc.sync.dma_start(rb[:], bc_ap)
                            nc.vector.tensor_tensor(
                                outT[rows, hp, 512 * qc:512 * (qc + 1)],
                                po[0:64, :], rb[:], MULT)

                for kp in range(K // 2):
                    pending.append(lambda kp=kp, f=out_slice: f(kp=kp))
                # any unflushed slices from the previous block
                while prev:
                    prev.pop(0)()
                if qc == 1:
                    # this hp's outT is complete once `pending` flushes;
                    # queue its E units as filler for the next blocks
                    for st in range(ST):
                        proj_units.append((hp + 1, (lambda hp=hp, st=st: emit_e_unit(hp, st))))
            flush_pending()
            drain_units(10**9)


    nc.compile()
    return nc


def _get_compiled():
    global _compiled
    if _compiled is None:
        _compiled = _build_nc()
    return _compiled


def kernel(x, W_attn, W_proj):
    from concourse.bass_utils import run_bass_kernel_spmd

    x = np.asarray(x, dtype=np.float32)
    W_attn = np.asarray(W_attn, dtype=np.float32)
    W_proj = np.asarray(W_proj, dtype=np.float32)

    xT = np.ascontiguousarray(np.transpose(x, (0, 2, 1)))  # [B, D, S]
    wq = np.ascontiguousarray(W_attn[:, 0:D]) * np.float32(0.125)
    wk = np.ascontiguousarray(W_attn[:, D:2 * D])
    wv = np.ascontiguousarray(W_attn[:, 2 * D:3 * D])
    masks = _build_masks()

    nc = _get_compiled()
    in_maps = [
        {"xT": xT[b], "wq": wq, "wk": wk, "wv": wv, "wp": W_proj, "masks": masks}
        for b in range(B)
    ]
    res = run_bass_kernel_spmd(nc, in_maps, list(range(NC_)))
    y = np.stack([res.results[b]["y"] for b in range(B)], axis=0)
    return y.astype(np.float32)


# revision 15
# speedup vs baseline: 1.5873x; 1.5873x over previous
"""Multi-head causal attention (B=8, S=1024, D=768, H=12) on 8 trn2 NeuronCores.

Strategy: data-parallel over batch (one batch element per core, no collectives).

Per-core dataflow (fp32r matmuls except A@V in bf16):
  - host passes x^T: Q^T/K^T via transposed projection (W stationary, x^T
    moving), V via natural projection (x^T stationary, W_v moving) -> no
    on-device transposes.
  - attention as S^T[k,q] = K @ Q^T per head; two heads (dh=64) packed into
    the 128-row PE array via row tiling, causal block-skip throughout.
  - softmax: exp on ScalarE straight out of PSUM ([128,1024] two-bank spans;
    1/8 scale folded into W_q host-side; no max-subtraction needed at these
    magnitudes); causal 0/1 bf16 mask multiply only on diagonal-crossing
    blocks; denominator free via a ones column appended to V (row 64 of the
    A@V PSUM); division folded into the PSUM->SBUF copy of A@V
    (fast reciprocal + DMA partition-broadcast through a DRAM scratch).
  - Q/K projection work-units are woven between attention tile groups so the
    PE fills exp-wait gaps and the HAM clock stays warm.
"""
import sys

if "/opt/trn_rl_repo" not in sys.path:
    sys.path.insert(0, "/opt/trn_rl_repo")

import numpy as np

B, S, D, H = 8, 1024, 768, 12
DH = 64
NC_ = 8
NT = D // 128    # 6
ST = S // 128    # 8
QC = S // 512    # 2
VPW = H * (DH + 1)  # 780

_compiled = None


def _build_masks():
    import ml_dtypes

    i = np.arange(128)[:, None, None]
    t = np.arange(4)[None, :, None]
    j = np.arange(512)[None, None, :]
    m = ((128 * t + i) <= j).astype(np.float32)
    return m.astype(ml_dtypes.bfloat16)


def _build_nc():
    import concourse.bass as bass
    import concourse.mybir as mybir
    import concourse.tile as tile
    from concourse import bacc

    F32 = mybir.dt.float32
    F32R = mybir.dt.float32r
    BF16 = mybir.dt.bfloat16
    AF = mybir.ActivationFunctionType
    MULT = mybir.AluOpType.mult

    nc = bacc.Bacc("TRN2", target_bir_lowering=False, debug=False)

    xT_d = nc.dram_tensor("xT", [D, S], F32, kind="ExternalInput")
    wq_d = nc.dram_tensor("wq", [D, D], F32, kind="ExternalInput")
    wk_d = nc.dram_tensor("wk", [D, D], F32, kind="ExternalInput")
    wv_d = nc.dram_tensor("wv", [D, D], F32, kind="ExternalInput")
    wp_d = nc.dram_tensor("wp", [D, D], F32, kind="ExternalInput")
    mask_d = nc.dram_tensor("masks", [128, 4, 512], BF16, kind="ExternalInput")
    y_d = nc.dram_tensor("y", [S, D], F32, kind="ExternalOutput")
    recip_d = nc.dram_tensor("recip_scratch", [H, QC, 512], F32)

    with tile.TileContext(nc) as tc:
        with (
            tc.tile_pool(name="static", bufs=1) as static,
            tc.tile_pool(name="w", bufs=12) as wpool,
            tc.tile_pool(name="pt", bufs=12) as ptpool,
            tc.tile_pool(name="small", bufs=2) as small,
            tc.tile_pool(name="rbp", bufs=3) as rbp,
            tc.tile_pool(name="y", bufs=2) as ypool,
            tc.tile_pool(name="psb", bufs=2, space="PSUM") as psb,
            tc.tile_pool(name="psproj", bufs=2, space="PSUM") as psproj,
            tc.tile_pool(name="pso", bufs=2, space="PSUM") as pso,
        ):
            # ---- persistent SBUF ----
            xT = static.tile([128, NT, S], F32R)
            qT = static.tile([128, NT, S], F32R)
            kT = static.tile([128, NT, S], F32R)
            vp = static.tile([128, ST, VPW], BF16)
            outT = static.tile([128, NT, S], F32R)
            msk = static.tile([128, 4, 512], BF16)

            for dc in range(NT):
                nc.sync.dma_start(xT[:, dc, :], xT_d[128 * dc:128 * (dc + 1), :].bitcast(F32R))
            nc.sync.dma_start(msk[:], mask_d[:])
            nc.vector.memset(vp[:], 1.0)

            # ---- stage C: v' = x @ W_v (natural layout) + ones cols ----
            wv_t = []
            for dc in range(NT):
                w = wpool.tile([128, D], F32R, tag="w")
                nc.sync.dma_start(w[:], wv_d[128 * dc:128 * (dc + 1), :].bitcast(F32R))
                wv_t.append(w)
            for st in range(ST):
                ps = psb.tile([128, 1024], F32, tag="big")
                for dc in range(NT):
                    nc.tensor.matmul(
                        ps[:, 0:512], xT[:, dc, 128 * st:128 * (st + 1)],
                        wv_t[dc][:, 0:512], start=(dc == 0), stop=(dc == NT - 1))
                for dc in range(NT):
                    nc.tensor.matmul(
                        ps[:, 512:768], xT[:, dc, 128 * st:128 * (st + 1)],
                        wv_t[dc][:, 512:768], start=(dc == 0), stop=(dc == NT - 1))
                dst = vp[:, st, :].rearrange("p (h e) -> p h e", e=DH + 1)
                nc.vector.tensor_copy(
                    out=dst[:, 0:8, 0:DH],
                    in_=ps[:, 0:512].rearrange("p (h d) -> p h d", d=DH))
                nc.vector.tensor_copy(
                    out=dst[:, 8:12, 0:DH],
                    in_=ps[:, 512:768].rearrange("p (h d) -> p h d", d=DH))

            # ---- Q^T/K^T projection work units (woven into attention) ----
            def emit_proj_unit(w_tiles, nt, dst, sc):
                ps = psproj.tile([128, 512], F32, tag="proj")
                for dc in range(NT):
                    nc.tensor.matmul(
                        ps[:],
                        w_tiles[dc][:, 128 * nt:128 * (nt + 1)],
                        xT[:, dc, 512 * sc:512 * (sc + 1)],
                        start=(dc == 0), stop=(dc == NT - 1))
                nc.vector.tensor_copy(out=dst[:, nt, 512 * sc:512 * (sc + 1)], in_=ps[:])

            proj_units = []

            def pop_unit():
                if proj_units:
                    proj_units.pop(0)[1]()

            def drain_units(hp_limit):
                while proj_units and proj_units[0][0] <= hp_limit:
                    proj_units.pop(0)[1]()

            wq_t, wk_t = [], []
            for dc in range(NT):
                w = wpool.tile([128, D], F32R, tag="w")
                nc.sync.dma_start(w[:], wq_d[128 * dc:128 * (dc + 1), :].bitcast(F32R))
                wq_t.append(w)
            for dc in range(NT):
                w = wpool.tile([128, D], F32R, tag="w")
                nc.sync.dma_start(w[:], wk_d[128 * dc:128 * (dc + 1), :].bitcast(F32R))
                wk_t.append(w)

            for hp in range(NT):
                for sc in range(2):
                    proj_units.append((hp, (lambda nt=hp, sc=sc: emit_proj_unit(wq_t, nt, qT, sc))))
                    proj_units.append((hp, (lambda nt=hp, sc=sc: emit_proj_unit(wk_t, nt, kT, sc))))

            # ---- attention blocks ----
            for hp in range(NT):
                drain_units(hp)

                for qc in range(QC):
                    K = 4 * (qc + 1)
                    pts = {0: [], 1: []}
                    for kp in range(K // 2):
                        tiles = {}
                        for hh in range(2):
                            t_ = psb.tile([128, 1024], F32, tag="big", name=f"st_{hp}_{qc}_{kp}_{hh}")
                            tiles[hh] = t_
                        for j in range(2):
                            kc = 2 * kp + j
                            for hh in range(2):
                                rows = slice(64 * hh, 64 * (hh + 1))
                                nc.tensor.matmul(
                                    tiles[hh][:, 512 * j:512 * (j + 1)],
                                    kT[rows, hp, 128 * kc:128 * (kc + 1)],
                                    qT[rows, hp, 512 * qc:512 * (qc + 1)],
                                    start=True, stop=True,
                                    tile_position=(64 * hh, 0))
                        for hh in range(2):
                            pt = ptpool.tile([128, 1024], BF16, tag="pt")
                            nc.scalar.activation(pt[:], tiles[hh][:], AF.Exp)
                            for j in range(2):
                                t = 2 * kp + j - 4 * qc
                                if 0 <= t <= 3:
                                    half = pt[:, 512 * j:512 * (j + 1)]
                                    nc.vector.tensor_tensor(half, half, msk[:, t, :], MULT)
                            pts[hh].append(pt)
                        pop_unit()

                    for hh in range(2):
                        h = 2 * hp + hh
                        rows = slice(64 * hh, 64 * (hh + 1))
                        po = pso.tile([65, 512], F32, tag="po")
                        for kc in range(K):
                            nc.tensor.matmul(
                                po[:],
                                vp[:, kc, 65 * h:65 * (h + 1)],
                                pts[hh][kc // 2][:, 512 * (kc % 2):512 * (kc % 2 + 1)],
                                start=(kc == 0), stop=(kc == K - 1))
                        den = small.tile([1, 512], F32, tag="den")
                        nc.vector.tensor_copy(out=den[:], in_=po[64:65, :])
                        rc = small.tile([1, 512], F32, tag="rc")
                        nc.vector.reciprocal_approx_fast(out=rc[:], in_=den[:])
                        nc.sync.dma_start(recip_d[h, qc, :], rc[:])
                        rb = rbp.tile([64, 512], F32, tag="rb")
                        sl = recip_d[h, qc, :]
                        bc_ap = bass.AP(tensor=sl.tensor, offset=sl.offset,
                                        ap=[[0, 64]] + list(sl.ap))
                        nc.sync.dma_start(rb[:], bc_ap)
                        nc.vector.tensor_tensor(
                            outT[rows, hp, 512 * qc:512 * (qc + 1)],
                            po[0:64, :], rb[:], MULT)

            # ---- stage E: y = out @ W_proj ----
            wp_t = []
            for dc in range(NT):
                w = wpool.tile([128, D], F32R, tag="w")
                nc.sync.dma_start(w[:], wp_d[128 * dc:128 * (dc + 1), :].bitcast(F32R))
                wp_t.append(w)
            for st in range(ST):
                ps = psb.tile([128, 1024], F32, tag="big")
                for dc in range(NT):
                    nc.tensor.matmul(
                        ps[:, 0:512], outT[:, dc, 128 * st:128 * (st + 1)],
                        wp_t[dc][:, 0:512], start=(dc == 0), stop=(dc == NT - 1))
                for dc in range(NT):
                    nc.tensor.matmul(
                        ps[:, 512:768], outT[:, dc, 128 * st:128 * (st + 1)],
                        wp_t[dc][:, 512:768], start=(dc == 0), stop=(dc == NT - 1))
                y_sb = ypool.tile([128, D], F32, tag="y")
                nc.vector.tensor_copy(out=y_sb[:], in_=ps[:, 0:768])
                nc.sync.dma_start(y_d[128 * st:128 * (st + 1), :], y_sb[:])

    nc.compile()
    return nc


def _get_compiled():
    global _compiled
    if _compiled is None:
        _compiled = _build_nc()
    return _compiled


def kernel(x, W_attn, W_proj):
    from concourse.bass_utils import run_bass_kernel_spmd

    x = np.asarray(x, dtype=np.float32)
    W_attn = np.asarray(W_attn, dtype=np.float32)
    W_proj = np.asarray(W_proj, dtype=np.float32)

    xT = np.ascontiguousarray(np.transpose(x, (0, 2, 1)))
    wq = np.ascontiguousarray(W_attn[:, 0:D]) * np.float32(0.125)
    wk = np.ascontiguousarray(W_attn[:, D:2 * D])
    wv = np.ascontiguousarray(W_attn[:, 2 * D:3 * D])
    masks = _build_masks()

    nc = _get_compiled()
    in_maps = [
        {"xT": xT[b], "wq": wq, "wk": wk, "wv": wv, "wp": W_proj, "masks": masks}
        for b in range(B)
    ]
    res = run_bass_kernel_spmd(nc, in_maps, list(range(NC_)))
    y = np.stack([res.results[b]["y"] for b in range(B)], axis=0)
    return y.astype(np.float32)
